# Initial kernel scaffold
#
"""Causal GQA self-attention on 8 Trainium2 NeuronCores.

Sharding: data-parallel over batch (4) x tensor-parallel over heads (2 halves
of 14 heads each, KV heads replicated for the shared GQA group). Each core
computes a partial output (its heads' contribution through the row-parallel
out-projection); the host sums the two partials per batch element.

Per-core head assignment is chosen so every core sees an identical local
structure (local heads 0..13, local kv-groups 0..3, quad q <-> group q):
  half 0: global heads [0..11, 24, 25],  kv heads [0, 1, 2, 6]
  half 1: global heads [12..23, 26, 27], kv heads [3, 4, 5, 6]
The host permutes weight columns/rows into this local order.

Kernel layout strategy (all SBUF tensors [128 partitions, free...]):
  xT  [128, 7, 2048] : x^T (C on partitions) via PE transpose
  QT  [128, 4, 2048] : Q^T, local head h at (partitions 32*(h%4), chunk h//4)
  KT  [128, 4, 2048] : K^T per local group, replicated on all 4 row slots
  V   [128, 16, 128] : V (kpos on partitions)
  AOT [128, 4, 2048] : attention output transposed (head dims on partitions)
Scores are computed transposed S^T[kpos, q] with 4 row-tiled (tile_position)
K=32 matmuls per quad; exp on ScalarE (PSUM->SBUF, scale folded in); P^T then
feeds col-tiled AV and Z(=sum) matmuls accumulating over kpos chunks; final
out-projection consumes AOT directly as the stationary operand.
"""

import sys

sys.path.insert(0, "/opt/trn_rl_repo")

from contextlib import ExitStack

import numpy as np

import concourse.bass as bass
import concourse.mybir as mybir
import concourse.tile as tile
from concourse import bacc
from concourse.bass import ts
from concourse.bass_utils import run_bass_kernel_spmd

F32 = mybir.dt.float32
F32R = mybir.dt.float32r
EXP = mybir.ActivationFunctionType.Exp
P = 128
T, C = 2048, 896
D = 32
HL = 14  # local heads per core
GL = 4  # local kv groups per core
DH = HL * D  # 448
DKV = GL * D  # 128
SCALE = 1.0 / float(np.sqrt(D))

HEADS_HALF = [
    list(range(0, 12)) + [24, 25],
    list(range(12, 24)) + [26, 27],
]
KV_HALF = [[0, 1, 2, 6], [3, 4, 5, 6]]


def _trace(tc, d):
    nc = tc.nc
    with ExitStack() as ctx:
        const = ctx.enter_context(tc.tile_pool(name="const", bufs=1))
        ident = const.tile([P, P], F32)
        nc.sync.dma_start(ident[:], d["ident"][:])
        maskb = const.tile([P, P], F32)
        nc.sync.dma_start(maskb[:], d["mask"][:])
        identr = const.tile([P, P], F32R)
        nc.sync.dma_start(identr[:], d["identr"][:])

        persist = ctx.enter_context(tc.tile_pool(name="persist", bufs=1))
        QT = persist.tile([P, 4, T], F32R, tag="QT")
        KT = persist.tile([P, 4, T], F32R, tag="KT")
        V = persist.tile([P, 16, GL, 64], F32R, tag="V")

        nc.sync.dma_start(
            V[:, :, :, D:64],
            d["vones"].rearrange("p (a b c) -> p a b c", a=16, b=GL),
        )

        with tc.tile_pool(name="ph01", bufs=1) as ph01:
            xT = ph01.tile([P, 7, T], F32R, tag="xT")
            # ------------- phase 0: x -> xT (PE transpose) -------------
            with tc.tile_pool(name="xraw", bufs=8) as xraw, \
                 tc.tile_pool(name="pst", bufs=2, space="PSUM") as pst:
                xv = d["x"].rearrange("(to ti) c -> ti to c", ti=P)
                for tcg in range(4):
                    xt4 = []
                    for k in range(4):
                        xtile = xraw.tile([P, C], F32, tag="xtile")
                        nc.sync.dma_start(xtile[:], xv[:, 4 * tcg + k, :])
                        xt4.append(xtile)
                    for cc in range(7):
                        ps = pst.tile([P, 512], F32, tag="tps")
                        for k in range(4):
                            nc.tensor.transpose(
                                ps[:, ts(k, P)], xt4[k][:, ts(cc, P)], ident[:]
                            )
                        nc.vector.tensor_copy(xT[:, cc, ts(tcg, 512)], ps[:])

            # ---------------- phase 1: projections ----------------
            with tc.tile_pool(name="w1", bufs=1) as w1, \
                 tc.tile_pool(name="vtt", bufs=2) as vtt, \
                 tc.tile_pool(name="pst2", bufs=2, space="PSUM") as pst2, \
                 tc.tile_pool(name="psp", bufs=2, space="PSUM") as psp:
                WqH = w1.tile([P, 7, DH], F32R, tag="WqH")
                nc.sync.dma_start(
                    WqH[:], d["wq"].rearrange("(co ci) n -> ci co n", ci=P)
                )
                WkR = w1.tile([P, 7, GL, P], F32R, tag="WkR")
                wkv = d["wk"].rearrange("(co ci) n -> ci co n", ci=P)
                for g in range(GL):
                    for i in range(4):
                        nc.sync.dma_start(
                            WkR[:, :, g, ts(i, D)], wkv[:, :, ts(g, D)]
                        )
                WvH = w1.tile([P, 7, DKV], F32R, tag="WvH")
                nc.sync.dma_start(
                    WvH[:], d["wv"].rearrange("(co ci) n -> ci co n", ci=P)
                )

                # QT: out[m=dim chunk, n=t] accumulate over C chunks
                for mc in range(4):
                    M = P if mc < 3 else 64
                    for nk in range(4):
                        ps = psp.tile([P, 512], F32, tag="pps")
                        for c in range(7):
                            nc.tensor.matmul(
                                ps[:M, :],
                                lhsT=WqH[:, c, mc * P : mc * P + M],
                                rhs=xT[:, c, ts(nk, 512)],
                                start=(c == 0),
                                stop=(c == 6),
                            )
                        nc.vector.tensor_copy(QT[:M, mc, ts(nk, 512)], ps[:M, :])
                # KT (replicated): per local group
                for g in range(GL):
                    for nk in range(4):
                        ps = psp.tile([P, 512], F32, tag="pps")
                        for c in range(7):
                            nc.tensor.matmul(
                                ps[:],
                                lhsT=WkR[:, c, g, :],
                                rhs=xT[:, c, ts(nk, 512)],
                                start=(c == 0),
                                stop=(c == 6),
                            )
                        nc.vector.tensor_copy(KT[:, g, ts(nk, 512)], ps[:])
                # VT then transpose to V
                for nk in range(4):
                    ps = psp.tile([P, 512], F32, tag="pps")
                    for c in range(7):
                        nc.tensor.matmul(
                            ps[:],
                            lhsT=WvH[:, c, :],
                            rhs=xT[:, c, ts(nk, 512)],
                            start=(c == 0),
                            stop=(c == 6),
                        )
                    vts = vtt.tile([P, 512], F32, tag="vts")
                    nc.vector.tensor_copy(vts[:], ps[:])
                    for k in range(4):
                        vps = pst2.tile([P, 512], F32, tag="tps")
                        nc.tensor.transpose(vps[:, :P], vts[:, ts(k, P)], ident[:])
                        nc.vector.tensor_copy(
                            V[:, nk * 4 + k, :, 0:D],
                            vps[:, :P].rearrange("p (g e) -> p g e", g=GL),
                        )

        # ---------------- phase 2+3: attention + out-proj ----------------
        with tc.tile_pool(name="w2", bufs=1) as w2, \
             tc.tile_pool(name="pts", bufs=2) as pts, \
             tc.tile_pool(name="ziP", bufs=2) as zip_, \
             tc.tile_pool(name="outs", bufs=2) as outs_p, \
             tc.tile_pool(name="pss", bufs=2, space="PSUM") as pss, \
             tc.tile_pool(name="psav", bufs=2, space="PSUM") as psav, \
             tc.tile_pool(name="pso", bufs=2, space="PSUM") as pso:
            AOT = w2.tile([P, 4, T], F32R, tag="AOT")
            WoH = w2.tile([P, 4, C], F32R, tag="WoH")
            nc.sync.dma_start(
                WoH[:, :3, :], d["wo"][: 3 * P, :].rearrange("(co ci) n -> ci co n", ci=P)
            )
            nc.sync.dma_start(WoH[:64, 3, :], d["wo"][3 * P :, :])
            ov = d["out"].rearrange("(to ti) c -> ti to c", ti=P)

            for qc in range(4):
                qs = qc * 512
                for pr in range(7):
                    h0 = 2 * pr
                    g = h0 // 4
                    j0 = h0 % 4
                    ava = psav.tile([64, 512], F32, tag="av")
                    avb = psav.tile([64, 512], F32, tag="av")
                    avs = [ava, avb]
                    nks = qs // P + 4
                    for ki in range(nks):
                        ks = ki * P
                        qoff = max(0, ks - qs)
                        pt = pts.tile([P, 2, 512], F32R, tag="pt")
                        sp = pss.tile([P, 2, 512], F32, tag="sp")
                        for j2 in range(2):
                            j = j0 + j2
                            nc.tensor.matmul(
                                sp[:, j2, qoff:512],
                                lhsT=KT[ts(j, D), g, ks : ks + P],
                                rhs=QT[ts(j, D), g, qs + qoff : qs + 512],
                                start=True,
                                stop=True,
                                tile_position=(j * D, 0),
                            )
                        nc.scalar.activation(
                            pt[:, :, qoff:512],
                            sp[:, :, qoff:512],
                            EXP,
                            scale=SCALE,
                        )
                        if ks >= qs:  # diagonal chunk: zero the triangle
                            nc.vector.tensor_tensor(
                                pt[:, :, qoff : qoff + P],
                                pt[:, :, qoff : qoff + P],
                                maskb[:, None, :].to_broadcast((P, 2, P)),
                                mybir.AluOpType.mult,
                            )
                        for j2 in range(2):
                            nc.tensor.matmul(
                                avs[j2][0:64, qoff:512],
                                lhsT=V[:, ki, g, 0:64],
                                rhs=pt[:, j2, qoff:512],
                                start=(ki == 0),
                                stop=(ki == nks - 1),
                                skip_group_check=True,
                            )
                    zq = pss.tile([P, 2, 512], F32, tag="sp")
                    for j2 in range(2):
                        h = h0 + j2
                        av = avs[j2]
                        zt = zip_.tile([64, 512], F32R, tag="zt")
                        nc.vector.tensor_copy(zt[D:64, :], av[D:64, :])
                        nc.tensor.matmul(
                            zq[0:D, j2, :],
                            lhsT=identr[D:64, D:64],
                            rhs=zt[D:64, :],
                            start=True,
                            stop=True,
                            tile_position=(D, 0),
                        )
                        zs = zip_.tile([D, 512], F32, tag="zs")
                        nc.vector.reciprocal_approx_fast(zs[:], zq[0:D, j2, :])
                        ao = zip_.tile([D, 512], F32R, tag="ao")
                        nc.vector.tensor_tensor(
                            ao[:],
                            av[0:D, :],
                            zs[:],
                            mybir.AluOpType.mult,
                        )
                        nc.sync.dma_start(
                            AOT[ts(h % 4, D), g, qs : qs + 512], ao[:]
                        )
                # out-projection for this q-chunk
                for tcl in range(4):
                    tg = qc * 4 + tcl
                    ob = outs_p.tile([P, C], F32, tag="ob")
                    for ncol in range(2):
                        po = pso.tile([P, 448], F32, tag="po")
                        for c in range(4):
                            K = P if c < 3 else 64
                            nc.tensor.matmul(
                                po[:],
                                lhsT=AOT[:K, c, qs + tcl * P : qs + (tcl + 1) * P],
                                rhs=WoH[:K, c, ncol * 448 : (ncol + 1) * 448],
                                start=(c == 0),
                                stop=(c == 3),
                            )
                        nc.vector.tensor_copy(ob[:, ncol * 448 : (ncol + 1) * 448], po[:])
                    nc.sync.dma_start(ov[:, tg, :], ob[:])


_NC_CACHE = None


def _build():
    global _NC_CACHE
    if _NC_CACHE is not None:
        return _NC_CACHE
    nc = bacc.Bacc("TRN2", target_bir_lowering=False, debug=False, num_devices=8)
    d = {
        "x": nc.dram_tensor("x", (T, C), F32, kind="ExternalInput"),
        "wq": nc.dram_tensor("wq", (C, DH), F32R, kind="ExternalInput"),
        "wk": nc.dram_tensor("wk", (C, DKV), F32R, kind="ExternalInput"),
        "wv": nc.dram_tensor("wv", (C, DKV), F32R, kind="ExternalInput"),
        "wo": nc.dram_tensor("wo", (DH, C), F32R, kind="ExternalInput"),
        "ident": nc.dram_tensor("ident", (P, P), F32, kind="ExternalInput"),
        "mask": nc.dram_tensor("mask", (P, P), F32, kind="ExternalInput"),
        "vones": nc.dram_tensor("vones", (P, 16 * GL * D), F32R, kind="ExternalInput"),
        "identr": nc.dram_tensor("identr", (P, P), F32R, kind="ExternalInput"),
        "out": nc.dram_tensor("out", (T, C), F32, kind="ExternalOutput"),

    }
    with tile.TileContext(nc) as tc:
        _trace(tc, {k: v[:] for k, v in d.items()})
    nc.compile()
    _NC_CACHE = nc
    return nc


def _in_maps(x, Wq, Wk, Wv, Wo):
    ident = np.eye(P, dtype=np.float32)
    vones = np.ones((P, 16 * GL * D), dtype=np.float32)
    maskb = (
        np.arange(P)[None, :] >= np.arange(P)[:, None]
    ).astype(np.float32)  # [kpos_p, q_j] valid when j >= p
    maps = []
    for c in range(8):
        b, hf = c // 2, c % 2
        hcols = np.concatenate([np.arange(32 * h, 32 * h + 32) for h in HEADS_HALF[hf]])
        kcols = np.concatenate([np.arange(32 * g, 32 * g + 32) for g in KV_HALF[hf]])
        maps.append(
            {
                "x": np.ascontiguousarray(x[b]),
                "wq": np.ascontiguousarray(Wq[:, hcols]),
                "wk": np.ascontiguousarray(Wk[:, kcols]),
                "wv": np.ascontiguousarray(Wv[:, kcols]),
                "wo": np.ascontiguousarray(Wo[hcols, :]),
                "ident": ident,
                "mask": maskb,
                "vones": vones,
                "identr": ident,
            }
        )
    return maps


def run(x, Wq, Wk, Wv, Wo, trace=False):
    nc = _build()
    res = run_bass_kernel_spmd(
        nc, _in_maps(x, Wq, Wk, Wv, Wo), core_ids=list(range(8)), trace=trace
    )
    outs = [r["out"] for r in res.results]
    final = np.empty((4, T, C), np.float32)
    for b in range(4):
        final[b] = outs[2 * b] + outs[2 * b + 1]
    return final, res


def kernel(x, Wq, Wk, Wv, Wo):
    x = np.asarray(x, dtype=np.float32)
    out, _ = run(
        x,
        np.asarray(Wq, np.float32),
        np.asarray(Wk, np.float32),
        np.asarray(Wv, np.float32),
        np.asarray(Wo, np.float32),
    )
    return out



# revision 7
# speedup vs baseline: 1.0819x; 1.0819x over previous
"""Causal GQA self-attention on 8 Trainium2 NeuronCores.

Sharding: data-parallel over batch (4) x tensor-parallel over heads (2 halves
of 14 heads each, KV heads replicated for the shared GQA group). Each core
computes a partial output (its heads' contribution through the row-parallel
out-projection); the host sums the two partials per batch element.

Local head layout (per core): 14 heads = 4 kv-groups x up-to-4 heads. Head
slot (g, c) lives at QT partition rows 32g..32g+32, chunk c; its kv group g's
K lives at KT rows 32g (no replication needed: Q and K share the partition
range, tile_position=(32g, 0)). Host permutes Wq columns / Wo rows into
(c-major, g-minor) order so chunk c of QT is a contiguous 128-column block.

Engine plan (cost model: engine time ~ output free-size, partitions free):
  PE:   x transpose (f32r), QKV proj, scores S^T[kpos, q] (bf16), causal mask
        via a -1e5 upper-tri matmul accumulated into the diagonal PSUM block
        pre-exp, AV as out[q, 33] (col 32 = softmax denominator via a ones
        column in V), out-projection.
  ACT:  exp only (PSUM -> SBUF bf16, scale folded).
  DVE:  xT/KT/vts PSUM evictions, reciprocal of denominators.
  Pool: QT evictions, AO normalize (av * 1/z), small memsets.
  DMA:  XBAR transposes for V and the attention output (SBUF->SBUF bf16).
"""

import sys

sys.path.insert(0, "/opt/trn_rl_repo")

from contextlib import ExitStack

import numpy as np

import concourse.bass as bass
import concourse.mybir as mybir
import concourse.tile as tile
from concourse import bacc
from concourse.bass import ts
from concourse.bass_utils import run_bass_kernel_spmd

F32 = mybir.dt.float32
F32R = mybir.dt.float32r
BF16 = mybir.dt.bfloat16
EXP = mybir.ActivationFunctionType.Exp
MUL = mybir.AluOpType.mult
P = 128
T, C = 2048, 896
D = 32
HL = 14  # local heads per core
GL = 4  # local kv groups per core
DH = HL * D  # 448
DKV = GL * D  # 128
SCALE = 1.0 / float(np.sqrt(D))
NEG = -1.0e5  # causal mask additive value (pre-scale)

HEADS_HALF = [
    list(range(0, 12)) + [24, 25],
    list(range(12, 24)) + [26, 27],
]
KV_HALF = [[0, 1, 2, 6], [3, 4, 5, 6]]
# head pairs (g, c0): heads (g, c0) and (g, c0+1)
PAIRS = [(0, 0), (0, 2), (1, 0), (1, 2), (2, 0), (2, 2), (3, 0)]
# rows used per chunk c (chunks 2,3 only have groups 0..2)
CH_ROWS = [128, 128, 96, 96]
# chunk column offsets within the 448 permuted head dims
CH_OFF = [0, 128, 256, 352]


def _trace(tc, d):
    nc = tc.nc
    with ExitStack() as ctx:
        const = ctx.enter_context(tc.tile_pool(name="const", bufs=1))
        identr = const.tile([P, P], F32R)
        nc.sync.dma_start(identr[:], d["identr"][:])
        identb = const.tile([P, P], BF16)
        nc.sync.dma_start(identb[:], d["identb"][:])
        maskT = const.tile([P, P], BF16)
        nc.sync.dma_start(maskT[:], d["masktb"][:])

        persist = ctx.enter_context(tc.tile_pool(name="persist", bufs=1))
        xT = persist.tile([P, 7, T], F32R, tag="xT")
        QT = persist.tile([P, 4, T], BF16, tag="QT")
        KT = persist.tile([P, T], BF16, tag="KT")
        V = persist.tile([P, 16, GL, 33], BF16, tag="V")
        AOT = persist.tile([P, 4, T], BF16, tag="AOT")

        w = ctx.enter_context(tc.tile_pool(name="w", bufs=1))
        WqH = w.tile([P, 7, DH], F32R, tag="WqH")
        nc.sync.dma_start(WqH[:], d["wq"].rearrange("(co ci) n -> ci co n", ci=P))
        WkH = w.tile([P, 7, DKV], F32R, tag="WkH")
        nc.sync.dma_start(WkH[:], d["wk"].rearrange("(co ci) n -> ci co n", ci=P))
        WvH = w.tile([P, 7, DKV], F32R, tag="WvH")
        nc.sync.dma_start(WvH[:], d["wv"].rearrange("(co ci) n -> ci co n", ci=P))
        WoH = w.tile([P, 4, C], BF16, tag="WoH")
        for c in range(4):
            nc.sync.dma_start(
                WoH[: CH_ROWS[c], c, :],
                d["wo"][CH_OFF[c] : CH_OFF[c] + CH_ROWS[c], :],
            )
        # ones column of V (softmax denominator accumulator)
        nc.gpsimd.memset(V[:, :, :, 32:33], 1.0)

        xv = d["x"].rearrange("(to ti) c -> ti to c", ti=P)
        ov = d["out"].rearrange("(to ti) c -> ti to c", ti=P)

        xraw = ctx.enter_context(tc.tile_pool(name="xraw", bufs=8))
        pp = ctx.enter_context(tc.tile_pool(name="pp", bufs=2, space="PSUM"))
        pss = ctx.enter_context(tc.tile_pool(name="pss", bufs=2, space="PSUM"))
        pav = ctx.enter_context(tc.tile_pool(name="pav", bufs=2, space="PSUM"))
        vtt = ctx.enter_context(tc.tile_pool(name="vtt", bufs=2))
        ptp = ctx.enter_context(tc.tile_pool(name="ptp", bufs=3))
        rzp = ctx.enter_context(tc.tile_pool(name="rzp", bufs=2))
        aop = ctx.enter_context(tc.tile_pool(name="aop", bufs=2))
        obp = ctx.enter_context(tc.tile_pool(name="obp", bufs=2))

        def out_proj(qc):
            qs = qc * 512
            for tcl in range(4):
                tg = qc * 4 + tcl
                for half in range(2):
                    po = pp.tile([P, 448], F32, tag="pp")
                    for c in range(4):
                        K = CH_ROWS[c]
                        nc.tensor.matmul(
                            po[:],
                            lhsT=AOT[:K, c, qs + tcl * P : qs + (tcl + 1) * P],
                            rhs=WoH[:K, c, half * 448 : (half + 1) * 448],
                            start=(c == 0),
                            stop=(c == 3),
                        )
                    ob = obp.tile([P, 448], F32, tag="ob")
                    nc.vector.tensor_copy(ob[:], po[:])
                    nc.sync.dma_start(ov[:, tg, half * 448 : (half + 1) * 448], ob[:])

        for step in range(4):
            nk = step
            # ---- x -> xT (PE transpose, f32r) ----
            xt4 = []
            for k in range(4):
                xtile = xraw.tile([P, C], F32R, tag="xtile")
                nc.sync.dma_start(xtile[:], xv[:, 4 * step + k, :])
                xt4.append(xtile)
            for cc in range(7):
                ps = pp.tile([P, 512], F32R, tag="pp")
                for k in range(4):
                    nc.tensor.transpose(ps[:, ts(k, P)], xt4[k][:, ts(cc, P)], identr[:])
                nc.vector.tensor_copy(xT[:, cc, ts(step, 512)], ps[:])

            # ---- projections for this t-chunk (nk) ----
            for c in range(4):
                M = CH_ROWS[c]
                ps = pp.tile([P, 512], F32, tag="pp")
                for cc in range(7):
                    nc.tensor.matmul(
                        ps[:M, :],
                        lhsT=WqH[:, cc, CH_OFF[c] : CH_OFF[c] + M],
                        rhs=xT[:, cc, ts(nk, 512)],
                        start=(cc == 0),
                        stop=(cc == 6),
                    )
                nc.vector.tensor_copy(QT[:M, c, ts(nk, 512)], ps[:M, :])
            ps = pp.tile([P, 512], F32, tag="pp")
            for cc in range(7):
                nc.tensor.matmul(
                    ps[:],
                    lhsT=WkH[:, cc, :],
                    rhs=xT[:, cc, ts(nk, 512)],
                    start=(cc == 0),
                    stop=(cc == 6),
                )
            nc.vector.tensor_copy(KT[:, ts(nk, 512)], ps[:])
            ps = pp.tile([P, 512], F32, tag="pp")
            for cc in range(7):
                nc.tensor.matmul(
                    ps[:],
                    lhsT=WvH[:, cc, :],
                    rhs=xT[:, cc, ts(nk, 512)],
                    start=(cc == 0),
                    stop=(cc == 6),
                )
            vts = vtt.tile([P, 512], BF16, tag="vts")
            nc.vector.tensor_copy(vts[:], ps[:])
            for k in range(4):
                nc.sync.dma_start(
                    V[:, 4 * nk + k, :, 0:32], vts[:, ts(k, P)], transpose=True
                )

            # ---- out-projection for the previous q-chunk ----
            if step >= 1:
                out_proj(step - 1)

            # ---- attention for q-chunk qc = step ----
            qc = step
            qs = qc * 512
            AOn = aop.tile([P, 4, 4, GL, D], BF16, tag="AOn")  # [q, qsub, c, g, d]
            # zero the unused (c>=2, g=3) rows so XBAR input is initialized
            nc.gpsimd.memset(AOn[:, :, 2, 3, :], 0.0)
            nc.gpsimd.memset(AOn[:, :, 3, 3, :], 0.0)
            for g, c0 in PAIRS:
                av = pav.tile([P, 2, 4, 33], F32, tag="av")
                nks = 4 * qc + 4
                for ki in range(nks):
                    ks = ki * P
                    qoff = max(0, ks - qs)
                    sp = pss.tile([P, 2, 512], F32, tag="sp")
                    for j2 in range(2):
                        nc.tensor.matmul(
                            sp[:, j2, qoff:512],
                            lhsT=KT[ts(g, D), ks : ks + P],
                            rhs=QT[ts(g, D), c0 + j2, qs + qoff : qs + 512],
                            start=True,
                            stop=True,
                            tile_position=(g * D, 0),
                        )
                    if ks >= qs:
                        for j2 in range(2):
                            nc.tensor.matmul(
                                sp[:, j2, qoff : qoff + P],
                                lhsT=maskT[:],
                                rhs=identb[:],
                                start=False,
                                stop=True,
                                skip_group_check=True,
                            )
                    pt = ptp.tile([P, 2, 512], BF16, tag="pt")
                    nc.scalar.activation(
                        pt[:, :, qoff:512], sp[:, :, qoff:512], EXP, scale=SCALE
                    )
                    q0 = max(0, ki - 4 * qc)
                    for j2 in range(2):
                        for qsub in range(q0, 4):
                            nc.tensor.matmul(
                                av[:, j2, qsub, :],
                                lhsT=pt[:, j2, ts(qsub, P)],
                                rhs=V[:, ki, g, :],
                                start=(ki == 0),
                                stop=(ki == 4 * qc + qsub),
                                skip_group_check=True,
                            )
                rz = rzp.tile([P, 2, 4], F32, tag="rz")
                nc.vector.reciprocal_approx_fast(rz[:], av[:, :, :, 32])
                for j2 in range(2):
                    nc.vector.tensor_tensor(
                        AOn[:, :, c0 + j2, g, :],
                        av[:, j2, :, 0:32],
                        rz[:, j2, :, None].to_broadcast((P, 4, D)),
                        MUL,
                    )
            # XBAR transpose AOn -> AOT
            for qsub in range(4):
                for c in range(4):
                    nc.sync.dma_start(
                        AOT[:, c, qs + qsub * P : qs + (qsub + 1) * P],
                        AOn[:, qsub, c, :, :],
                        transpose=True,
                    )
        out_proj(3)


_NC_CACHE = None


def _build():
    global _NC_CACHE
    if _NC_CACHE is not None:
        return _NC_CACHE
    nc = bacc.Bacc("TRN2", target_bir_lowering=False, debug=False, num_devices=8)
    d = {
        "x": nc.dram_tensor("x", (T, C), F32R, kind="ExternalInput"),
        "wq": nc.dram_tensor("wq", (C, DH), F32R, kind="ExternalInput"),
        "wk": nc.dram_tensor("wk", (C, DKV), F32R, kind="ExternalInput"),
        "wv": nc.dram_tensor("wv", (C, DKV), F32R, kind="ExternalInput"),
        "wo": nc.dram_tensor("wo", (DH, C), BF16, kind="ExternalInput"),
        "identr": nc.dram_tensor("identr", (P, P), F32R, kind="ExternalInput"),
        "identb": nc.dram_tensor("identb", (P, P), BF16, kind="ExternalInput"),
        "masktb": nc.dram_tensor("masktb", (P, P), BF16, kind="ExternalInput"),
        "out": nc.dram_tensor("out", (T, C), F32, kind="ExternalOutput"),
    }
    with tile.TileContext(nc) as tc:
        _trace(tc, {k: v[:] for k, v in d.items()})
    nc.compile()
    _NC_CACHE = nc
    return nc


def _head_cols(hf):
    """Permuted head order: c-major, g-minor (head (g,c) -> global head id)."""
    order = []
    for c in range(4):
        for g in range(4):
            if g == 3 and c >= 2:
                continue
            if g < 3:
                gh = HEADS_HALF[hf][4 * g + c]
            else:
                gh = HEADS_HALF[hf][12 + c]
            order.append(gh)
    return np.concatenate([np.arange(32 * h, 32 * h + 32) for h in order])


def _in_maps(x, Wq, Wk, Wv, Wo):
    import ml_dtypes

    bf16 = np.dtype(ml_dtypes.bfloat16)
    identr = np.eye(P, dtype=np.float32)
    identb_bits = np.eye(P, dtype=np.float32).astype(bf16)
    masktb = np.where(
        np.arange(P)[:, None] < np.arange(P)[None, :], np.float32(NEG), np.float32(0)
    )
    masktb_bits = masktb.astype(bf16)
    maps = []
    for cidx in range(8):
        b, hf = cidx // 2, cidx % 2
        hcols = _head_cols(hf)
        kcols = np.concatenate([np.arange(32 * g, 32 * g + 32) for g in KV_HALF[hf]])
        maps.append(
            {
                "x": np.ascontiguousarray(x[b]),
                "wq": np.ascontiguousarray(Wq[:, hcols]),
                "wk": np.ascontiguousarray(Wk[:, kcols]),
                "wv": np.ascontiguousarray(Wv[:, kcols]),
                "wo": np.ascontiguousarray(Wo[hcols, :]).astype(bf16),
                "identr": identr,
                "identb": identb_bits,
                "masktb": masktb_bits,
            }
        )
    return maps


def run(x, Wq, Wk, Wv, Wo, trace=False):
    nc = _build()
    res = run_bass_kernel_spmd(
        nc, _in_maps(x, Wq, Wk, Wv, Wo), core_ids=list(range(8)), trace=trace
    )
    outs = [r["out"] for r in res.results]
    final = np.empty((4, T, C), np.float32)
    for b in range(4):
        final[b] = outs[2 * b] + outs[2 * b + 1]
    return final, res


def kernel(x, Wq, Wk, Wv, Wo):
    x = np.asarray(x, dtype=np.float32)
    out, _ = run(
        x,
        np.asarray(Wq, np.float32),
        np.asarray(Wk, np.float32),
        np.asarray(Wv, np.float32),
        np.asarray(Wo, np.float32),
    )
    return out
